# revision 1
# baseline (speedup 1.0000x reference)
"""Multi-head self-attention Bass/Tile kernel for Trainium2, SPMD over 8 cores.

Problem: B=2, T=4096, D=768, H=12, HD=64 dense MHSA (full TxT scores,
key-padding mask, softmax, out-proj with bias).

Sharding: core c handles batch b=c//4 and query slice q0=(c%4)*1024 for all
12 heads over the full 4096 keys.  No collectives: each core computes a
disjoint [768, 1024] slice of the (transposed) output; the host gathers.

All matmuls contract over the partition dim, so the dataflow is "transposed"
(features on partitions, tokens free):
  phase A: QKV projection.  Q^T per head [64, 1024] stays in SBUF;
           K^T [768, 4096] and V' [12, 4096, 65] staged via DRAM
           (V' carries a ones column per head -> softmax denominator
           falls out of the AV matmul).
  phase B: per head h, per key-tile kt: S[128k, 1024q] = K_h^T.T @ Q_h^T,
           P = exp(S/8 + maskbias_k) on ACT (mask is a per-partition bias),
           O'[65, 512] += V'_kt.T @ P (PSUM accumulation over 32 key tiles).
           Normalize O = O'[0:64] * bcast(1/O'[64]) (PE broadcast matmul).
  phase C: out^T[768, 1024] = Wp^T.T @ O^T + b, DMA out.
"""

import functools
import numpy as np

import concourse.bass as bass
import concourse.mybir as mybir
import concourse.tile as tile
from concourse import bacc
from concourse.bass2jax import (
    _bass_exec_p,
    install_neuronx_cc_hook,
    partition_id_tensor,
)

F32R = mybir.dt.float32r
F32 = mybir.dt.float32
BF16 = mybir.dt.bfloat16
USE_BF16 = True
MMDT = BF16 if USE_BF16 else F32R
AF = mybir.ActivationFunctionType

B, T, D = 2, 4096, 768
H, HD = 12, 64
N_CORES = 8
CORES_PER_B = 4
QS = T // CORES_PER_B          # 1024 query tokens per core
NB = 1e9                        # mask bias magnitude
DT = D // 128                   # 6 d-tiles
KT = T // 128                   # 32 key tiles
QC = QS // 512                  # 2 query chunks of 512


def build_program(reps: int = 1):
    nc = bacc.Bacc("TRN2", target_bir_lowering=False, debug=False,
                   num_devices=N_CORES)

    xT = nc.dram_tensor("xT", [D, T], MMDT, kind="ExternalInput").ap()
    xTq = nc.dram_tensor("xTq", [D, QS], MMDT, kind="ExternalInput").ap()
    wqT = nc.dram_tensor("wqT", [D, D], MMDT, kind="ExternalInput").ap()
    wkT = nc.dram_tensor("wkT", [D, D], MMDT, kind="ExternalInput").ap()
    wvT = nc.dram_tensor("wvT", [D, D], MMDT, kind="ExternalInput").ap()
    wpT = nc.dram_tensor("wpT", [D, D], MMDT, kind="ExternalInput").ap()
    bp = nc.dram_tensor("bp", [128, DT], F32, kind="ExternalInput").ap()
    mb = nc.dram_tensor("mb", [128, KT], F32, kind="ExternalInput").ap()
    onesc = nc.dram_tensor("onesc", [128, H], MMDT, kind="ExternalInput").ap()
    outT = nc.dram_tensor("outT", [D, QS], F32, kind="ExternalOutput").ap()

    KTd = nc.dram_tensor("KTd", [D, T], MMDT).ap()          # K^T staging
    Vp = nc.dram_tensor("Vp", [T, H * (HD + 1)], MMDT).ap()  # V' staging

    with tile.TileContext(nc) as tc, nc.allow_low_precision(
            reason="f32r matmul pipeline"):
        _body(nc, tc, reps, xT, xTq, wqT, wkT, wvT, wpT, bp, mb, onesc,
              outT, KTd, Vp)
    nc.compile()
    return nc


def _body(nc, tc, reps, xT, xTq, wqT, wkT, wvT, wpT, bp, mb, onesc,
          outT, KTd, Vp):
    from contextlib import ExitStack

    with ExitStack() as root:
        const = root.enter_context(tc.tile_pool(name="const", bufs=1))
        mb_sb = const.tile([128, KT], F32, tag="mb")
        nc.sync.dma_start(mb_sb[:], mb[:])
        bp_sb = const.tile([128, DT], F32, tag="bp")
        nc.sync.dma_start(bp_sb[:], bp[:])
        ones64 = const.tile([1, 64], F32, tag="ones64")
        nc.vector.memset(ones64[:], 1.0)
        onesr = const.tile([128, H], MMDT, tag="onesr")
        nc.sync.dma_start(onesr[:], onesc[:])

        # long-lived per-head Q^T and O^T
        qt_pool = root.enter_context(tc.tile_pool(name="qt", bufs=1))
        ot_pool = root.enter_context(tc.tile_pool(name="ot", bufs=1))

        def emit_once():
            qts = _phase_a(nc, tc, qt_pool, xT, xTq, wqT, wkT, wvT, onesr,
                           KTd, Vp)
            ots = _phase_b(nc, tc, ot_pool, qts, mb_sb, ones64, KTd, Vp)
            _phase_c(nc, tc, ots, wpT, bp_sb, outT)

        if reps == 1:
            emit_once()
        elif reps < 0:
            for _ in range(-reps):
                emit_once()
        else:
            with tc.For_i(0, reps, 1):
                emit_once()


def _phase_a(nc, tc, qt_pool, xT, xTq, wqT, wkT, wvT, onesr, KTd, Vp):
    from contextlib import ExitStack

    # --- Q^T projection: per-head tiles [64, QS], SBUF-resident ---
    qts = []
    with ExitStack() as s:
        wq_pool = s.enter_context(tc.tile_pool(name="wq", bufs=1))
        xq_pool = s.enter_context(tc.tile_pool(name="xq", bufs=1))
        qps_pool = s.enter_context(
            tc.tile_pool(name="qps", bufs=2, space="PSUM"))

        wq_sb, xq_sb = [], []
        for d in range(DT):
            w = wq_pool.tile([128, D], MMDT, tag=f"wq{d}")
            nc.sync.dma_start(w[:], wqT[d * 128:(d + 1) * 128, :])
            wq_sb.append(w)
            xq = xq_pool.tile([128, QS], MMDT, tag=f"xq{d}")
            nc.sync.dma_start(xq[:], xTq[d * 128:(d + 1) * 128, :])
            xq_sb.append(xq)

        for h in range(H):
            qt = qt_pool.tile([64, QS], MMDT, tag=f"qt{h}")
            for c in range(QC):
                ps = qps_pool.tile([64, 512], F32, tag="qps")
                for d in range(DT):
                    nc.tensor.matmul(
                        ps[:], wq_sb[d][:, h * 64:(h + 1) * 64],
                        xq_sb[d][:, c * 512:(c + 1) * 512],
                        start=(d == 0), stop=(d == DT - 1))
                nc.vector.tensor_copy(qt[:, c * 512:(c + 1) * 512], ps[:])
            qts.append(qt)

    # --- K^T and V' over the full T, staged to DRAM ---
    with ExitStack() as s:
        wkv_pool = s.enter_context(tc.tile_pool(name="wkv", bufs=1))
        xt_pool = s.enter_context(tc.tile_pool(name="xt", bufs=2))
        stage_pool = s.enter_context(tc.tile_pool(name="stage", bufs=3))
        kps_pool = s.enter_context(
            tc.tile_pool(name="kps", bufs=2, space="PSUM"))
        vps_pool = s.enter_context(
            tc.tile_pool(name="vps", bufs=2, space="PSUM"))

        wk_sb, wv_sb = [], []
        for d in range(DT):
            wk = wkv_pool.tile([128, D], MMDT, tag=f"wk{d}")
            nc.sync.dma_start(wk[:], wkT[d * 128:(d + 1) * 128, :])
            wk_sb.append(wk)
            wv = wkv_pool.tile([128, D], MMDT, tag=f"wv{d}")
            nc.sync.dma_start(wv[:], wvT[d * 128:(d + 1) * 128, :])
            wv_sb.append(wv)

        for tch in range(T // 1024):
            tsl = slice(tch * 1024, (tch + 1) * 1024)
            xt_sb = []
            for d in range(DT):
                xt_t = xt_pool.tile([128, 1024], MMDT, tag=f"xt{d}")
                nc.sync.dma_start(xt_t[:], xT[d * 128:(d + 1) * 128, tsl])
                xt_sb.append(xt_t)

            # K^T rows e*128..e*128+128, cols tsl
            for e in range(DT):
                kst = stage_pool.tile([128, 1024], MMDT, tag="kst")
                for half in range(2):
                    hs = slice(half * 512, (half + 1) * 512)
                    ps = kps_pool.tile([128, 512], F32, tag="kps")
                    for d in range(DT):
                        nc.tensor.matmul(
                            ps[:], wk_sb[d][:, e * 128:(e + 1) * 128],
                            xt_sb[d][:, hs],
                            start=(d == 0), stop=(d == DT - 1))
                    nc.vector.tensor_copy(kst[:, hs], ps[:])
                nc.sync.dma_start(KTd[e * 128:(e + 1) * 128, tsl], kst[:])

            # V natural layout [t, e] + ones col per head
            for tt in range(8):
                t0 = tch * 1024 + tt * 128
                ps = vps_pool.tile([128, D], F32, tag="vps")
                for d in range(DT):
                    lhs = xt_sb[d][:, tt * 128:(tt + 1) * 128]
                    nc.tensor.matmul(ps[:, 0:512], lhs, wv_sb[d][:, 0:512],
                                     start=(d == 0), stop=(d == DT - 1),
                                     skip_group_check=True)
                    nc.tensor.matmul(ps[:, 512:768], lhs, wv_sb[d][:, 512:768],
                                     start=(d == 0), stop=(d == DT - 1),
                                     skip_group_check=True)
                vst = stage_pool.tile([128, H * (HD + 1)], MMDT, tag="vst")
                vst3 = vst[:].rearrange("p (h s) -> p h s", s=HD + 1)
                nc.vector.tensor_copy(
                    vst3[:, :, 0:HD],
                    ps[:].rearrange("p (h s) -> p h s", s=HD))
                nc.vector.tensor_copy(
                    vst3[:, :, HD:HD + 1],
                    onesr[:].rearrange("p (h o) -> p h o", o=1))
                nc.sync.dma_start(Vp[t0:t0 + 128, :], vst[:])
    return qts


def _phase_b(nc, tc, ot_pool, qts, mb_sb, ones64, KTd, Vp):
    from contextlib import ExitStack

    ots = []
    with ExitStack() as s:
        kh_pool = s.enter_context(tc.tile_pool(name="kh", bufs=2))
        vh_pool = s.enter_context(tc.tile_pool(name="vh", bufs=2))
        p_pool = s.enter_context(tc.tile_pool(name="p", bufs=3))
        nrm_pool = s.enter_context(tc.tile_pool(name="nrm", bufs=2))
        sp_pool = s.enter_context(
            tc.tile_pool(name="sp", bufs=2, space="PSUM"))
        op_pool = s.enter_context(
            tc.tile_pool(name="op", bufs=1, space="PSUM"))
        bc_pool = s.enter_context(
            tc.tile_pool(name="bc", bufs=1, space="PSUM"))

        for h in range(H):
            kh = kh_pool.tile([64, T], MMDT, tag="kh")
            nc.sync.dma_start(kh[:], KTd[h * 64:(h + 1) * 64, :])
            vh = vh_pool.tile([128, KT * (HD + 1)], MMDT, tag="vh")
            nc.sync.dma_start(
                vh[:].rearrange("p (kt s) -> p kt s", s=HD + 1),
                Vp.rearrange("(kt p) (h s) -> p kt h s", p=128,
                             s=HD + 1)[:, :, h, :])

            ops = [op_pool.tile([65, 512], F32, tag=f"op{c}", name=f"op{c}")
                   for c in range(QC)]
            for kt in range(KT):
                sp = sp_pool.tile([128, QC * 512], F32, tag="sp")
                for c in range(QC):
                    nc.tensor.matmul(
                        sp[:, c * 512:(c + 1) * 512],
                        kh[:, kt * 128:(kt + 1) * 128],
                        qts[h][:, c * 512:(c + 1) * 512],
                        start=True, stop=True, skip_group_check=True)
                p = p_pool.tile([128, QC * 512], MMDT, tag="p")
                nc.scalar.activation(p[:], sp[:], AF.Exp,
                                     bias=mb_sb[:, kt:kt + 1], scale=0.125)
                for c in range(QC):
                    nc.tensor.matmul(
                        ops[c][:],
                        vh[:, kt * (HD + 1):(kt + 1) * (HD + 1)],
                        p[:, c * 512:(c + 1) * 512],
                        start=(kt == 0), stop=(kt == KT - 1))

            ot = ot_pool.tile([64, QS], MMDT, tag=f"ot{h}")
            for c in range(QC):
                recip = nrm_pool.tile([1, 512], F32, tag="recip")
                nc.vector.reciprocal(recip[:], ops[c][64:65, :])
                bc = bc_pool.tile([64, 512], F32, tag="bc")
                nc.tensor.matmul(bc[:], ones64[:], recip[:],
                                 start=True, stop=True)
                bc_sb = nrm_pool.tile([64, 512], F32, tag="bc_sb")
                nc.vector.tensor_copy(bc_sb[:], bc[:])
                nc.vector.tensor_mul(ot[:, c * 512:(c + 1) * 512],
                                     ops[c][0:64, :], bc_sb[:])
            ots.append(ot)
    return ots


def _phase_c(nc, tc, ots, wpT, bp_sb, outT):
    from contextlib import ExitStack

    with ExitStack() as s:
        wp_pool = s.enter_context(tc.tile_pool(name="wp", bufs=1))
        ost_pool = s.enter_context(tc.tile_pool(name="ost", bufs=3))
        pps_pool = s.enter_context(
            tc.tile_pool(name="pps", bufs=2, space="PSUM"))

        wp_sb = []
        for h in range(H):
            wp = wp_pool.tile([64, D], MMDT, tag=f"wp{h}")
            nc.sync.dma_start(wp[:], wpT[h * 64:(h + 1) * 64, :])
            wp_sb.append(wp)

        for m in range(DT):
            for c in range(QC):
                ps = pps_pool.tile([128, 512], F32, tag="pps")
                for h in range(H):
                    nc.tensor.matmul(
                        ps[:], wp_sb[h][:, m * 128:(m + 1) * 128],
                        ots[h][:, c * 512:(c + 1) * 512],
                        start=(h == 0), stop=(h == H - 1))
                ost = ost_pool.tile([128, 512], F32, tag="ost")
                nc.vector.tensor_scalar_add(ost[:], ps[:], bp_sb[:, m:m + 1])
                nc.sync.dma_start(
                    outT[m * 128:(m + 1) * 128, c * 512:(c + 1) * 512],
                    ost[:])


# ---------------------------------------------------------------- host side

@functools.lru_cache(maxsize=None)
def _get_runner(reps: int = 1):
    import jax
    from jax.sharding import Mesh, PartitionSpec
    from jax.experimental.shard_map import shard_map

    nc = build_program(reps)
    install_neuronx_cc_hook()
    partition_name = (nc.partition_id_tensor.name
                      if nc.partition_id_tensor else None)
    in_names, out_names, out_avals, out_shapes = [], [], [], []
    for alloc in nc.m.functions[0].allocations:
        if not isinstance(alloc, mybir.MemoryLocationSet):
            continue
        name = alloc.memorylocations[0].name
        if alloc.kind == "ExternalInput":
            if name != partition_name:
                in_names.append(name)
        elif alloc.kind == "ExternalOutput":
            out_names.append(name)
            shape = tuple(alloc.tensor_shape)
            dtype = mybir.dt.np(alloc.dtype)
            out_avals.append(jax.core.ShapedArray(shape, dtype))
            out_shapes.append((shape, dtype))
    n_params = len(in_names)
    n_outs = len(out_avals)
    all_in_names = list(in_names) + list(out_names)
    if partition_name is not None:
        all_in_names.append(partition_name)
    donate = tuple(range(n_params, n_params + n_outs))

    def _bodyf(*args):
        operands = list(args)
        if partition_name is not None:
            operands.append(partition_id_tensor())
        outs = _bass_exec_p.bind(
            *operands,
            out_avals=tuple(out_avals),
            in_names=tuple(all_in_names),
            out_names=tuple(out_names),
            lowering_input_output_aliases=(),
            sim_require_finite=True,
            sim_require_nnan=True,
            nc=nc,
        )
        return tuple(outs)

    devices = jax.devices()[:N_CORES]
    mesh = Mesh(np.asarray(devices), ("core",))
    in_specs = (PartitionSpec("core"),) * (n_params + n_outs)
    out_specs = (PartitionSpec("core"),) * len(out_names)
    sharded = jax.jit(
        shard_map(_bodyf, mesh=mesh, in_specs=in_specs, out_specs=out_specs,
                  check_rep=False),
        donate_argnums=donate, keep_unused=True,
    )

    def run(in_maps):
        import jax as _jax
        per_core = [[np.asarray(m[n]) for n in in_names] for m in in_maps]
        concat_in = [np.concatenate([per_core[c][i] for c in range(N_CORES)],
                                    axis=0) for i in range(n_params)]
        concat_zeros = [np.zeros((N_CORES * s[0], *s[1:]), dt)
                        for (s, dt) in out_shapes]
        out_arrs = sharded(*concat_in, *concat_zeros)
        _jax.block_until_ready(out_arrs)
        return [
            {name: np.asarray(out_arrs[i]).reshape(
                N_CORES, *out_shapes[i][0])[c]
             for i, name in enumerate(out_names)}
            for c in range(N_CORES)
        ]

    return run


def make_in_maps(x, mask, w_qkv, w_proj, b_proj):
    import ml_dtypes
    mm_np = ml_dtypes.bfloat16 if USE_BF16 else np.float32
    x = np.asarray(x, np.float32)
    mask = np.asarray(mask)
    w_qkv = np.asarray(w_qkv, np.float32)
    w_proj = np.asarray(w_proj, np.float32)
    b_proj = np.asarray(b_proj, np.float32)

    wqT = np.ascontiguousarray(w_qkv[0:D].T).astype(mm_np)
    wkT = np.ascontiguousarray(w_qkv[D:2 * D].T).astype(mm_np)
    wvT = np.ascontiguousarray(w_qkv[2 * D:3 * D].T).astype(mm_np)
    wpT = np.ascontiguousarray(w_proj.T).astype(mm_np)
    bp = np.ascontiguousarray(b_proj.reshape(DT, 128).T)
    onesc = np.ones((128, H), mm_np)

    xTs = [np.ascontiguousarray(x[b].T).astype(mm_np) for b in range(B)]
    mbs = [np.ascontiguousarray(
        np.where(mask[b], np.float32(-NB), np.float32(0.0))
        .astype(np.float32).reshape(KT, 128).T) for b in range(B)]

    in_maps = []
    for c in range(N_CORES):
        b, qi = divmod(c, CORES_PER_B)
        q0 = qi * QS
        in_maps.append({
            "xT": xTs[b],
            "xTq": np.ascontiguousarray(xTs[b][:, q0:q0 + QS]),
            "wqT": wqT, "wkT": wkT, "wvT": wvT, "wpT": wpT,
            "bp": bp, "mb": mbs[b], "onesc": onesc,
        })
    return in_maps


def assemble_output(results):
    out = np.empty((B, T, D), np.float32)
    for c in range(N_CORES):
        b, qi = divmod(c, CORES_PER_B)
        q0 = qi * QS
        out[b, q0:q0 + QS, :] = results[c]["outT"].T
    return out


def kernel(x, mask, w_qkv, w_proj, b_proj):
    run = _get_runner(1)
    in_maps = make_in_maps(x, mask, w_qkv, w_proj, b_proj)
    results = run(in_maps)
    return assemble_output(results)



# revision 6
# speedup vs baseline: 9.9761x; 9.9761x over previous
"""Multi-head self-attention Bass/Tile kernel for Trainium2, SPMD over 8 cores.

Problem: B=2, T=4096, D=768, H=12, HD=64 dense MHSA (full TxT scores,
key-padding mask, softmax, out-proj with bias).

Sharding: core c handles batch b=c//4 and query slice q0=(c%4)*1024 for all
12 heads.  No collectives: each core computes a disjoint [768, 1024] slice
of the (transposed) output; the host gathers.

Key optimization: the boolean key-padding mask knocks out ~half the keys and
masked keys contribute exactly zero to both softmax numerator and
denominator (exp(-inf)=0).  The host therefore compacts the key axis to the
~2048 valid keys (padded to a multiple of 128) before launching, halving the
score matmuls, the exp work, and the AV matmuls.  Padding columns of the
compacted x are zero, so S=0 -> exp(0)=1 for pad rows, and the ones-column
of V' (which yields the softmax denominator through the AV matmul) is zeroed
for pad rows, so pads contribute exactly nothing.

Dataflow (transposed: features on partitions, tokens free):
  phase A: QKV projection, all SBUF-resident.  Q^T/K^T in head-PAIR tiles
           [128, *] (head 2j on partitions 0-63, head 2j+1 on 64-127) so
           projection matmuls contract full 128-partition tiles; V' natural
           [t, e] layout with a ones column per head.
  phase B: per head h, per key-tile kt: S^T[128k, 1024q] = K_h^T.T @ Q_h^T
           (K=64 contraction via PE row-tiles; odd heads use base partition
           64), P = exp(S/8) on ACT, O'[65, 512] += V'_kt.T @ P with PSUM
           accumulation over key tiles.  Normalize via DVE reciprocal +
           GPSIMD partition_broadcast + DVE multiply.
  phase C: out^T[768, 1024] = Wp^T.T @ O^T + b, DMA out.

Phase A work is interleaved into phase B's emission so the PE stream stays
dense while ACT paces the softmax.
"""

import functools
import numpy as np

import concourse.bass as bass
import concourse.mybir as mybir
import concourse.tile as tile
from concourse import bacc
from concourse.bass2jax import (
    _bass_exec_p,
    install_neuronx_cc_hook,
    partition_id_tensor,
)

F32 = mybir.dt.float32
BF16 = mybir.dt.bfloat16
AF = mybir.ActivationFunctionType

B, T, D = 2, 4096, 768
H, HD = 12, 64
N_CORES = 8
CORES_PER_B = 4
QS = T // CORES_PER_B          # 1024 query tokens per core
DT = D // 128                   # 6 d-tiles
HP = H // 2                     # 6 head pairs
QC = QS // 512                  # 2 query chunks of 512
MAX_TEFF = 2304                 # SBUF-resident limit for the compacted path


def compact_teff(mask):
    """Padded compacted key count shared by both batches."""
    mask = np.asarray(mask)
    cnts = [int((~mask[b]).sum()) for b in range(B)]
    teff = max(1, max(cnts))
    teff = (teff + 127) // 128 * 128
    return teff, cnts


# ---------------------------------------------------------------- program

def build_program(teff: int, reps: int = 1):
    nc = bacc.Bacc("TRN2", target_bir_lowering=False, debug=False,
                   num_devices=N_CORES)

    xqT = nc.dram_tensor("xqT", [D, QS], BF16, kind="ExternalInput").ap()
    xkT = nc.dram_tensor("xkT", [D, teff], BF16, kind="ExternalInput").ap()
    wqT = nc.dram_tensor("wqT", [D, D], BF16, kind="ExternalInput").ap()
    wkT = nc.dram_tensor("wkT", [D, D], BF16, kind="ExternalInput").ap()
    wvT = nc.dram_tensor("wvT", [D, D], BF16, kind="ExternalInput").ap()
    wpT = nc.dram_tensor("wpT", [D, D], BF16, kind="ExternalInput").ap()
    bp = nc.dram_tensor("bp", [128, DT], F32, kind="ExternalInput").ap()
    onesv = nc.dram_tensor("onesv", [128, teff // 128], BF16,
                           kind="ExternalInput").ap()
    outT = nc.dram_tensor("outT", [D, QS], F32, kind="ExternalOutput").ap()

    with tile.TileContext(nc) as tc, nc.allow_low_precision(
            reason="bf16 matmul pipeline"):
        def emit_once():
            _emit(nc, tc, teff, xqT, xkT, wqT, wkT, wvT, wpT, bp, onesv,
                  outT)
        if reps == 1:
            emit_once()
        elif reps < 0:
            for _ in range(-reps):
                emit_once()
        else:
            with tc.For_i(0, reps, 1):
                emit_once()
    nc.compile()
    return nc


def _emit(nc, tc, teff, xqT, xkT, wqT, wkT, wvT, wpT, bp, onesv, outT):
    from contextlib import ExitStack

    KT = teff // 128
    kchunks = [(o, min(512, teff - o)) for o in range(0, teff, 512)]

    with ExitStack() as root:
        def pool(name, bufs, space="SBUF"):
            return root.enter_context(
                tc.tile_pool(name=name, bufs=bufs, space=space))

        const = pool("const", 1)
        wpool = pool("w", 1)
        xpool = pool("x", 1)
        qt_pool = pool("qt", 1)
        kt_pool = pool("kt", 1)
        vp_pool = pool("vp", 1)
        ot_pool = pool("ot", 1)
        p_pool = pool("p", 3)
        osb_pool = pool("osb", 2)
        rc_pool = pool("rc", 2)
        bc_pool = pool("bc", 2)
        ost_pool = pool("ost", 2)
        sp_pool = pool("sp", 2, space="PSUM")       # 2 x [128,1024] = 4 banks
        op_pool = pool("op", 1, space="PSUM")       # 2 x [65, 512]  = 2 banks
        aps_pool = pool("aps", 2, space="PSUM")     # 2 x [128, 512] = 2 banks

        # --- input DMAs, in consumption order ---
        bp_sb = const.tile([128, DT], F32, tag="bp")
        onesv_sb = const.tile([128, KT], BF16, tag="onesv")
        wq_sb, wk_sb, wv_sb, wp_sb, xq_sb, xk_sb = [], [], [], [], [], []
        for d in range(DT):
            w = wpool.tile([128, D], BF16, tag=f"wq{d}")
            nc.sync.dma_start(w[:], wqT[d * 128:(d + 1) * 128, :])
            wq_sb.append(w)
        for d in range(DT):
            xq = xpool.tile([128, QS], BF16, tag=f"xq{d}")
            nc.sync.dma_start(xq[:], xqT[d * 128:(d + 1) * 128, :])
            xq_sb.append(xq)
        for d in range(DT):
            w = wpool.tile([128, D], BF16, tag=f"wk{d}")
            nc.sync.dma_start(w[:], wkT[d * 128:(d + 1) * 128, :])
            wk_sb.append(w)
        for d in range(DT):
            xk = xpool.tile([128, teff], BF16, tag=f"xk{d}")
            nc.sync.dma_start(xk[:], xkT[d * 128:(d + 1) * 128, :])
            xk_sb.append(xk)
        for d in range(DT):
            w = wpool.tile([128, D], BF16, tag=f"wv{d}")
            nc.sync.dma_start(w[:], wvT[d * 128:(d + 1) * 128, :])
            wv_sb.append(w)
        nc.sync.dma_start(onesv_sb[:], onesv[:])
        nc.sync.dma_start(bp_sb[:], bp[:])
        for d in range(DT):
            w = wpool.tile([128, D], BF16, tag=f"wp{d}")
            nc.sync.dma_start(w[:], wpT[d * 128:(d + 1) * 128, :])
            wp_sb.append(w)

        # --- long-lived SBUF tensors ---
        qts = [qt_pool.tile([128, QS], BF16, tag=f"qt{j}", name=f"qt{j}")
               for j in range(HP)]
        kts = [kt_pool.tile([128, teff], BF16, tag=f"kt{j}", name=f"kt{j}")
               for j in range(HP)]
        vp = vp_pool.tile([128, H * KT * 65], BF16, tag="vp")
        vp3 = vp[:].rearrange("p (h r) -> p h r", h=H)
        ots = [ot_pool.tile([128, QS], BF16, tag=f"ot{j}", name=f"ot{j}")
               for j in range(HP)]

        vp4 = vp[:].rearrange("p (h kt s) -> p h kt s", h=H, s=65)
        # ones columns of V' (denominator source; zero on pad rows)
        for h in range(H):
            nc.vector.tensor_copy(
                vp4[:, h:h + 1, :, 64:65],
                onesv_sb[:].rearrange("p (o kt u) -> p o kt u", o=1, u=1))

        # --- phase A emitters ---
        def emit_qproj(j):
            for c in range(QC):
                ps = aps_pool.tile([128, 512], F32, tag="aps")
                for d in range(DT):
                    nc.tensor.matmul(
                        ps[:], wq_sb[d][:, j * 128:(j + 1) * 128],
                        xq_sb[d][:, c * 512:(c + 1) * 512],
                        start=(d == 0), stop=(d == DT - 1))
                nc.vector.tensor_copy(qts[j][:, c * 512:(c + 1) * 512], ps[:])

        def emit_kproj(j, off, wdt):
            ps = aps_pool.tile([128, 512], F32, tag="aps")
            for d in range(DT):
                nc.tensor.matmul(
                    ps[:, 0:wdt], wk_sb[d][:, j * 128:(j + 1) * 128],
                    xk_sb[d][:, off:off + wdt],
                    start=(d == 0), stop=(d == DT - 1))
            nc.vector.tensor_copy(kts[j][:, off:off + wdt], ps[:, 0:wdt])

        def emit_vproj(kt):
            t0 = kt * 128
            ps1 = aps_pool.tile([128, 512], F32, tag="aps")
            for d in range(DT):
                nc.tensor.matmul(ps1[:], xk_sb[d][:, t0:t0 + 128],
                                 wv_sb[d][:, 0:512],
                                 start=(d == 0), stop=(d == DT - 1))
            nc.vector.tensor_copy(
                vp3[:, 0:8, kt * 65:kt * 65 + 64],
                ps1[:].rearrange("p (h s) -> p h s", s=64))
            ps2 = aps_pool.tile([128, 512], F32, tag="aps")
            for d in range(DT):
                nc.tensor.matmul(ps2[:, 0:256], xk_sb[d][:, t0:t0 + 128],
                                 wv_sb[d][:, 512:768],
                                 start=(d == 0), stop=(d == DT - 1))
            nc.vector.tensor_copy(
                vp3[:, 8:12, kt * 65:kt * 65 + 64],
                ps2[:, 0:256].rearrange("p (h s) -> p h s", s=64))

        # pending phase-A work, interleaved into phase B's emission
        pend = {h: [] for h in range(H)}
        for j in range(1, HP):
            items = [lambda j=j: emit_qproj(j)]
            items += [lambda j=j, o=o, w=w: emit_kproj(j, o, w)
                      for (o, w) in kchunks]
            pend[2 * j - 1] = items

        # --- prefix: pair 0 projections ---
        emit_qproj(0)
        for (o, w) in kchunks:
            emit_kproj(0, o, w)

        # --- phase B ---
        for h in range(H):
            j, hh = divmod(h, 2)
            lo, hi = hh * 64, hh * 64 + 64
            ops = [op_pool.tile([65, 512], F32, tag=f"op{c}", name=f"op{c}")
                   for c in range(QC)]
            for kt in range(KT):
                if h == 0:
                    emit_vproj(kt)
                elif pend[h] and kt % 3 == 1:
                    pend[h].pop(0)()
                sp = sp_pool.tile([128, 1024], F32, tag="sp")
                for c in range(QC):
                    nc.tensor.matmul(
                        sp[:, c * 512:(c + 1) * 512],
                        kts[j][lo:hi, kt * 128:(kt + 1) * 128],
                        qts[j][lo:hi, c * 512:(c + 1) * 512],
                        start=True, stop=True, skip_group_check=True)
                p = p_pool.tile([128, 1024], BF16, tag="p")
                nc.scalar.activation(p[:], sp[:], AF.Exp, scale=0.125)
                vslice = vp[:, (h * KT + kt) * 65:(h * KT + kt + 1) * 65]
                for c in range(QC):
                    nc.tensor.matmul(
                        ops[c][:], vslice,
                        p[:, c * 512:(c + 1) * 512],
                        start=(kt == 0), stop=(kt == KT - 1))
            while pend[h]:
                pend[h].pop(0)()
            # normalize: O = O'[0:64] / O'[64]
            o_sb = osb_pool.tile([65, QS], F32, tag="osb")
            for c in range(QC):
                nc.vector.tensor_copy(o_sb[:, c * 512:(c + 1) * 512],
                                      ops[c][:])
            rc = rc_pool.tile([1, QS], F32, tag="rc")
            nc.vector.reciprocal(rc[:], o_sb[64:65, :])
            bc = bc_pool.tile([64, QS], F32, tag="bc")
            nc.gpsimd.partition_broadcast(bc[:], rc[:])
            nc.vector.tensor_mul(ots[j][lo:hi, :], o_sb[0:64, :], bc[:])

        # --- phase C ---
        for m in range(DT):
            for c in range(QC):
                ps = aps_pool.tile([128, 512], F32, tag="aps")
                for j in range(HP):
                    nc.tensor.matmul(
                        ps[:], wp_sb[j][:, m * 128:(m + 1) * 128],
                        ots[j][:, c * 512:(c + 1) * 512],
                        start=(j == 0), stop=(j == HP - 1))
                ost = ost_pool.tile([128, 512], F32, tag="ost")
                nc.vector.tensor_scalar_add(ost[:], ps[:], bp_sb[:, m:m + 1])
                nc.sync.dma_start(
                    outT[m * 128:(m + 1) * 128, c * 512:(c + 1) * 512],
                    ost[:])


# ---------------------------------------------------------------- host side

@functools.lru_cache(maxsize=None)
def _get_runner(teff: int, reps: int = 1):
    import jax
    from jax.sharding import Mesh, PartitionSpec
    from jax.experimental.shard_map import shard_map

    nc = build_program(teff, reps)
    install_neuronx_cc_hook()
    partition_name = (nc.partition_id_tensor.name
                      if nc.partition_id_tensor else None)
    in_names, out_names, out_avals, out_shapes = [], [], [], []
    for alloc in nc.m.functions[0].allocations:
        if not isinstance(alloc, mybir.MemoryLocationSet):
            continue
        name = alloc.memorylocations[0].name
        if alloc.kind == "ExternalInput":
            if name != partition_name:
                in_names.append(name)
        elif alloc.kind == "ExternalOutput":
            out_names.append(name)
            shape = tuple(alloc.tensor_shape)
            dtype = mybir.dt.np(alloc.dtype)
            out_avals.append(jax.core.ShapedArray(shape, dtype))
            out_shapes.append((shape, dtype))
    n_params = len(in_names)
    n_outs = len(out_avals)
    all_in_names = list(in_names) + list(out_names)
    if partition_name is not None:
        all_in_names.append(partition_name)
    donate = tuple(range(n_params, n_params + n_outs))

    def _bodyf(*args):
        operands = list(args)
        if partition_name is not None:
            operands.append(partition_id_tensor())
        outs = _bass_exec_p.bind(
            *operands,
            out_avals=tuple(out_avals),
            in_names=tuple(all_in_names),
            out_names=tuple(out_names),
            lowering_input_output_aliases=(),
            sim_require_finite=True,
            sim_require_nnan=True,
            nc=nc,
        )
        return tuple(outs)

    devices = jax.devices()[:N_CORES]
    mesh = Mesh(np.asarray(devices), ("core",))
    in_specs = (PartitionSpec("core"),) * (n_params + n_outs)
    out_specs = (PartitionSpec("core"),) * len(out_names)
    sharded = jax.jit(
        shard_map(_bodyf, mesh=mesh, in_specs=in_specs, out_specs=out_specs,
                  check_rep=False),
        donate_argnums=donate, keep_unused=True,
    )

    def run(in_maps):
        import jax as _jax
        per_core = [[np.asarray(m[n]) for n in in_names] for m in in_maps]
        concat_in = [np.concatenate([per_core[c][i] for c in range(N_CORES)],
                                    axis=0) for i in range(n_params)]
        concat_zeros = [np.zeros((N_CORES * s[0], *s[1:]), dt)
                        for (s, dt) in out_shapes]
        out_arrs = sharded(*concat_in, *concat_zeros)
        _jax.block_until_ready(out_arrs)
        return [
            {name: np.asarray(out_arrs[i]).reshape(
                N_CORES, *out_shapes[i][0])[c]
             for i, name in enumerate(out_names)}
            for c in range(N_CORES)
        ]

    return run


def make_in_maps(x, mask, w_qkv, w_proj, b_proj):
    import ml_dtypes
    x = np.asarray(x, np.float32)
    mask = np.asarray(mask)
    w_qkv = np.asarray(w_qkv, np.float32)
    w_proj = np.asarray(w_proj, np.float32)
    b_proj = np.asarray(b_proj, np.float32)

    teff, cnts = compact_teff(mask)
    KT = teff // 128

    wqT = np.ascontiguousarray(w_qkv[0:D].T).astype(ml_dtypes.bfloat16)
    wkT = np.ascontiguousarray(w_qkv[D:2 * D].T).astype(ml_dtypes.bfloat16)
    wvT = np.ascontiguousarray(w_qkv[2 * D:3 * D].T).astype(ml_dtypes.bfloat16)
    wpT = np.ascontiguousarray(w_proj.T).astype(ml_dtypes.bfloat16)
    bp = np.ascontiguousarray(b_proj.reshape(DT, 128).T)

    xkTs, onesvs, xTs = [], [], []
    for b in range(B):
        idx = np.nonzero(~mask[b])[0]
        xk = np.zeros((teff, D), np.float32)
        xk[:cnts[b]] = x[b][idx]
        xkTs.append(np.ascontiguousarray(xk.T).astype(ml_dtypes.bfloat16))
        ones = (np.arange(teff) < cnts[b]).astype(np.float32)
        onesvs.append(np.ascontiguousarray(
            ones.reshape(KT, 128).T).astype(ml_dtypes.bfloat16))
        xTs.append(np.ascontiguousarray(x[b].T).astype(ml_dtypes.bfloat16))

    in_maps = []
    for c in range(N_CORES):
        b, qi = divmod(c, CORES_PER_B)
        q0 = qi * QS
        in_maps.append({
            "xqT": np.ascontiguousarray(xTs[b][:, q0:q0 + QS]),
            "xkT": xkTs[b],
            "wqT": wqT, "wkT": wkT, "wvT": wvT, "wpT": wpT,
            "bp": bp, "onesv": onesvs[b],
        })
    return in_maps


def assemble_output(results):
    out = np.empty((B, T, D), np.float32)
    for c in range(N_CORES):
        b, qi = divmod(c, CORES_PER_B)
        q0 = qi * QS
        out[b, q0:q0 + QS, :] = results[c]["outT"].T
    return out


def kernel(x, mask, w_qkv, w_proj, b_proj):
    teff, _ = compact_teff(mask)
    if teff > MAX_TEFF:
        return _fb_kernel(x, mask, w_qkv, w_proj, b_proj)
    run = _get_runner(teff, 1)
    in_maps = make_in_maps(x, mask, w_qkv, w_proj, b_proj)
    results = run(in_maps)
    return assemble_output(results)


# ------------------------------------------------------------------------
# Fallback path (no compaction; DRAM-staged K/V) for masks whose compacted
# key count does not fit the SBUF-resident layout.  This is the previous
# known-good kernel, kept verbatim.
# ------------------------------------------------------------------------

F32R = mybir.dt.float32r
MMDT = BF16
NB = 1e9
KTF = T // 128
FB_QC = QS // 512


def fb_build_program(reps: int = 1):
    nc = bacc.Bacc("TRN2", target_bir_lowering=False, debug=False,
                   num_devices=N_CORES)

    xT = nc.dram_tensor("xT", [D, T], MMDT, kind="ExternalInput").ap()
    xTq = nc.dram_tensor("xTq", [D, QS], MMDT, kind="ExternalInput").ap()
    wqT = nc.dram_tensor("wqT", [D, D], MMDT, kind="ExternalInput").ap()
    wkT = nc.dram_tensor("wkT", [D, D], MMDT, kind="ExternalInput").ap()
    wvT = nc.dram_tensor("wvT", [D, D], MMDT, kind="ExternalInput").ap()
    wpT = nc.dram_tensor("wpT", [D, D], MMDT, kind="ExternalInput").ap()
    bp = nc.dram_tensor("bp", [128, DT], F32, kind="ExternalInput").ap()
    mb = nc.dram_tensor("mb", [128, KTF], F32, kind="ExternalInput").ap()
    onesc = nc.dram_tensor("onesc", [128, H], MMDT, kind="ExternalInput").ap()
    outT = nc.dram_tensor("outT", [D, QS], F32, kind="ExternalOutput").ap()

    KTd = nc.dram_tensor("KTd", [D, T], MMDT).ap()
    Vp = nc.dram_tensor("Vp", [T, H * (HD + 1)], MMDT).ap()

    with tile.TileContext(nc) as tc, nc.allow_low_precision(
            reason="f32r matmul pipeline"):
        _fb_body(nc, tc, reps, xT, xTq, wqT, wkT, wvT, wpT, bp, mb, onesc,
                 outT, KTd, Vp)
    nc.compile()
    return nc


def _fb_body(nc, tc, reps, xT, xTq, wqT, wkT, wvT, wpT, bp, mb, onesc,
             outT, KTd, Vp):
    from contextlib import ExitStack

    with ExitStack() as root:
        const = root.enter_context(tc.tile_pool(name="const", bufs=1))
        mb_sb = const.tile([128, KTF], F32, tag="mb")
        nc.sync.dma_start(mb_sb[:], mb[:])
        bp_sb = const.tile([128, DT], F32, tag="bp")
        nc.sync.dma_start(bp_sb[:], bp[:])
        ones64 = const.tile([1, 64], F32, tag="ones64")
        nc.vector.memset(ones64[:], 1.0)
        onesr = const.tile([128, H], MMDT, tag="onesr")
        nc.sync.dma_start(onesr[:], onesc[:])

        qt_pool = root.enter_context(tc.tile_pool(name="qt", bufs=1))
        ot_pool = root.enter_context(tc.tile_pool(name="ot", bufs=1))

        def emit_once():
            qts = _fb_phase_a(nc, tc, qt_pool, xT, xTq, wqT, wkT, wvT, onesr,
                              KTd, Vp)
            ots = _fb_phase_b(nc, tc, ot_pool, qts, mb_sb, ones64, KTd, Vp)
            _fb_phase_c(nc, tc, ots, wpT, bp_sb, outT)

        if reps == 1:
            emit_once()
        elif reps < 0:
            for _ in range(-reps):
                emit_once()
        else:
            with tc.For_i(0, reps, 1):
                emit_once()


def _fb_phase_a(nc, tc, qt_pool, xT, xTq, wqT, wkT, wvT, onesr, KTd, Vp):
    from contextlib import ExitStack

    qts = []
    with ExitStack() as s:
        wq_pool = s.enter_context(tc.tile_pool(name="wq", bufs=1))
        xq_pool = s.enter_context(tc.tile_pool(name="xq", bufs=1))
        qps_pool = s.enter_context(
            tc.tile_pool(name="qps", bufs=2, space="PSUM"))

        wq_sb, xq_sb = [], []
        for d in range(DT):
            w = wq_pool.tile([128, D], MMDT, tag=f"wq{d}")
            nc.sync.dma_start(w[:], wqT[d * 128:(d + 1) * 128, :])
            wq_sb.append(w)
            xq = xq_pool.tile([128, QS], MMDT, tag=f"xq{d}")
            nc.sync.dma_start(xq[:], xTq[d * 128:(d + 1) * 128, :])
            xq_sb.append(xq)

        for h in range(H):
            qt = qt_pool.tile([64, QS], MMDT, tag=f"qt{h}")
            for c in range(FB_QC):
                ps = qps_pool.tile([64, 512], F32, tag="qps")
                for d in range(DT):
                    nc.tensor.matmul(
                        ps[:], wq_sb[d][:, h * 64:(h + 1) * 64],
                        xq_sb[d][:, c * 512:(c + 1) * 512],
                        start=(d == 0), stop=(d == DT - 1))
                nc.vector.tensor_copy(qt[:, c * 512:(c + 1) * 512], ps[:])
            qts.append(qt)

    with ExitStack() as s:
        wkv_pool = s.enter_context(tc.tile_pool(name="wkv", bufs=1))
        xt_pool = s.enter_context(tc.tile_pool(name="xt", bufs=2))
        stage_pool = s.enter_context(tc.tile_pool(name="stage", bufs=3))
        kps_pool = s.enter_context(
            tc.tile_pool(name="kps", bufs=2, space="PSUM"))
        vps_pool = s.enter_context(
            tc.tile_pool(name="vps", bufs=2, space="PSUM"))

        wk_sb, wv_sb = [], []
        for d in range(DT):
            wk = wkv_pool.tile([128, D], MMDT, tag=f"wk{d}")
            nc.sync.dma_start(wk[:], wkT[d * 128:(d + 1) * 128, :])
            wk_sb.append(wk)
            wv = wkv_pool.tile([128, D], MMDT, tag=f"wv{d}")
            nc.sync.dma_start(wv[:], wvT[d * 128:(d + 1) * 128, :])
            wv_sb.append(wv)

        for tch in range(T // 1024):
            tsl = slice(tch * 1024, (tch + 1) * 1024)
            xt_sb = []
            for d in range(DT):
                xt_t = xt_pool.tile([128, 1024], MMDT, tag=f"xt{d}")
                nc.sync.dma_start(xt_t[:], xT[d * 128:(d + 1) * 128, tsl])
                xt_sb.append(xt_t)

            for e in range(DT):
                kst = stage_pool.tile([128, 1024], MMDT, tag="kst")
                for half in range(2):
                    hs = slice(half * 512, (half + 1) * 512)
                    ps = kps_pool.tile([128, 512], F32, tag="kps")
                    for d in range(DT):
                        nc.tensor.matmul(
                            ps[:], wk_sb[d][:, e * 128:(e + 1) * 128],
                            xt_sb[d][:, hs],
                            start=(d == 0), stop=(d == DT - 1))
                    nc.vector.tensor_copy(kst[:, hs], ps[:])
                nc.sync.dma_start(KTd[e * 128:(e + 1) * 128, tsl], kst[:])

            for tt in range(8):
                t0 = tch * 1024 + tt * 128
                ps = vps_pool.tile([128, D], F32, tag="vps")
                for d in range(DT):
                    lhs = xt_sb[d][:, tt * 128:(tt + 1) * 128]
                    nc.tensor.matmul(ps[:, 0:512], lhs, wv_sb[d][:, 0:512],
                                     start=(d == 0), stop=(d == DT - 1),
                                     skip_group_check=True)
                    nc.tensor.matmul(ps[:, 512:768], lhs, wv_sb[d][:, 512:768],
                                     start=(d == 0), stop=(d == DT - 1),
                                     skip_group_check=True)
                vst = stage_pool.tile([128, H * (HD + 1)], MMDT, tag="vst")
                vst3 = vst[:].rearrange("p (h s) -> p h s", s=HD + 1)
                nc.vector.tensor_copy(
                    vst3[:, :, 0:HD],
                    ps[:].rearrange("p (h s) -> p h s", s=HD))
                nc.vector.tensor_copy(
                    vst3[:, :, HD:HD + 1],
                    onesr[:].rearrange("p (h o) -> p h o", o=1))
                nc.sync.dma_start(Vp[t0:t0 + 128, :], vst[:])
    return qts


def _fb_phase_b(nc, tc, ot_pool, qts, mb_sb, ones64, KTd, Vp):
    from contextlib import ExitStack

    ots = []
    with ExitStack() as s:
        kh_pool = s.enter_context(tc.tile_pool(name="kh", bufs=2))
        vh_pool = s.enter_context(tc.tile_pool(name="vh", bufs=2))
        p_pool = s.enter_context(tc.tile_pool(name="p", bufs=3))
        nrm_pool = s.enter_context(tc.tile_pool(name="nrm", bufs=2))
        sp_pool = s.enter_context(
            tc.tile_pool(name="sp", bufs=2, space="PSUM"))
        op_pool = s.enter_context(
            tc.tile_pool(name="op", bufs=1, space="PSUM"))
        bc_pool = s.enter_context(
            tc.tile_pool(name="bc", bufs=1, space="PSUM"))

        for h in range(H):
            kh = kh_pool.tile([64, T], MMDT, tag="kh")
            nc.sync.dma_start(kh[:], KTd[h * 64:(h + 1) * 64, :])
            vh = vh_pool.tile([128, KTF * (HD + 1)], MMDT, tag="vh")
            nc.sync.dma_start(
                vh[:].rearrange("p (kt s) -> p kt s", s=HD + 1),
                Vp.rearrange("(kt p) (h s) -> p kt h s", p=128,
                             s=HD + 1)[:, :, h, :])

            ops = [op_pool.tile([65, 512], F32, tag=f"op{c}", name=f"op{c}")
                   for c in range(FB_QC)]
            for kt in range(KTF):
                sp = sp_pool.tile([128, FB_QC * 512], F32, tag="sp")
                for c in range(FB_QC):
                    nc.tensor.matmul(
                        sp[:, c * 512:(c + 1) * 512],
                        kh[:, kt * 128:(kt + 1) * 128],
                        qts[h][:, c * 512:(c + 1) * 512],
                        start=True, stop=True, skip_group_check=True)
                p = p_pool.tile([128, FB_QC * 512], MMDT, tag="p")
                nc.scalar.activation(p[:], sp[:], AF.Exp,
                                     bias=mb_sb[:, kt:kt + 1], scale=0.125)
                for c in range(FB_QC):
                    nc.tensor.matmul(
                        ops[c][:],
                        vh[:, kt * (HD + 1):(kt + 1) * (HD + 1)],
                        p[:, c * 512:(c + 1) * 512],
                        start=(kt == 0), stop=(kt == KTF - 1))

            ot = ot_pool.tile([64, QS], MMDT, tag=f"ot{h}")
            for c in range(FB_QC):
                recip = nrm_pool.tile([1, 512], F32, tag="recip")
                nc.vector.reciprocal(recip[:], ops[c][64:65, :])
                bc = bc_pool.tile([64, 512], F32, tag="bc")
                nc.tensor.matmul(bc[:], ones64[:], recip[:],
                                 start=True, stop=True)
                bc_sb = nrm_pool.tile([64, 512], F32, tag="bc_sb")
                nc.vector.tensor_copy(bc_sb[:], bc[:])
                nc.vector.tensor_mul(ot[:, c * 512:(c + 1) * 512],
                                     ops[c][0:64, :], bc_sb[:])
            ots.append(ot)
    return ots


def _fb_phase_c(nc, tc, ots, wpT, bp_sb, outT):
    from contextlib import ExitStack

    with ExitStack() as s:
        wp_pool = s.enter_context(tc.tile_pool(name="wp", bufs=1))
        ost_pool = s.enter_context(tc.tile_pool(name="ost", bufs=3))
        pps_pool = s.enter_context(
            tc.tile_pool(name="pps", bufs=2, space="PSUM"))

        wp_sb = []
        for h in range(H):
            wp = wp_pool.tile([64, D], MMDT, tag=f"wp{h}")
            nc.sync.dma_start(wp[:], wpT[h * 64:(h + 1) * 64, :])
            wp_sb.append(wp)

        for m in range(DT):
            for c in range(FB_QC):
                ps = pps_pool.tile([128, 512], F32, tag="pps")
                for h in range(H):
                    nc.tensor.matmul(
                        ps[:], wp_sb[h][:, m * 128:(m + 1) * 128],
                        ots[h][:, c * 512:(c + 1) * 512],
                        start=(h == 0), stop=(h == H - 1))
                ost = ost_pool.tile([128, 512], F32, tag="ost")
                nc.vector.tensor_scalar_add(ost[:], ps[:], bp_sb[:, m:m + 1])
                nc.sync.dma_start(
                    outT[m * 128:(m + 1) * 128, c * 512:(c + 1) * 512],
                    ost[:])


@functools.lru_cache(maxsize=None)
def _fb_get_runner(reps: int = 1):
    import jax
    from jax.sharding import Mesh, PartitionSpec
    from jax.experimental.shard_map import shard_map

    nc = fb_build_program(reps)
    install_neuronx_cc_hook()
    partition_name = (nc.partition_id_tensor.name
                      if nc.partition_id_tensor else None)
    in_names, out_names, out_avals, out_shapes = [], [], [], []
    for alloc in nc.m.functions[0].allocations:
        if not isinstance(alloc, mybir.MemoryLocationSet):
            continue
        name = alloc.memorylocations[0].name
        if alloc.kind == "ExternalInput":
            if name != partition_name:
                in_names.append(name)
        elif alloc.kind == "ExternalOutput":
            out_names.append(name)
            shape = tuple(alloc.tensor_shape)
            dtype = mybir.dt.np(alloc.dtype)
            out_avals.append(jax.core.ShapedArray(shape, dtype))
            out_shapes.append((shape, dtype))
    n_params = len(in_names)
    n_outs = len(out_avals)
    all_in_names = list(in_names) + list(out_names)
    if partition_name is not None:
        all_in_names.append(partition_name)
    donate = tuple(range(n_params, n_params + n_outs))

    def _bodyf(*args):
        operands = list(args)
        if partition_name is not None:
            operands.append(partition_id_tensor())
        outs = _bass_exec_p.bind(
            *operands,
            out_avals=tuple(out_avals),
            in_names=tuple(all_in_names),
            out_names=tuple(out_names),
            lowering_input_output_aliases=(),
            sim_require_finite=True,
            sim_require_nnan=True,
            nc=nc,
        )
        return tuple(outs)

    devices = jax.devices()[:N_CORES]
    mesh = Mesh(np.asarray(devices), ("core",))
    in_specs = (PartitionSpec("core"),) * (n_params + n_outs)
    out_specs = (PartitionSpec("core"),) * len(out_names)
    sharded = jax.jit(
        shard_map(_bodyf, mesh=mesh, in_specs=in_specs, out_specs=out_specs,
                  check_rep=False),
        donate_argnums=donate, keep_unused=True,
    )

    def run(in_maps):
        import jax as _jax
        per_core = [[np.asarray(m[n]) for n in in_names] for m in in_maps]
        concat_in = [np.concatenate([per_core[c][i] for c in range(N_CORES)],
                                    axis=0) for i in range(n_params)]
        concat_zeros = [np.zeros((N_CORES * s[0], *s[1:]), dt)
                        for (s, dt) in out_shapes]
        out_arrs = sharded(*concat_in, *concat_zeros)
        _jax.block_until_ready(out_arrs)
        return [
            {name: np.asarray(out_arrs[i]).reshape(
                N_CORES, *out_shapes[i][0])[c]
             for i, name in enumerate(out_names)}
            for c in range(N_CORES)
        ]

    return run


def fb_make_in_maps(x, mask, w_qkv, w_proj, b_proj):
    import ml_dtypes
    mm_np = ml_dtypes.bfloat16
    x = np.asarray(x, np.float32)
    mask = np.asarray(mask)
    w_qkv = np.asarray(w_qkv, np.float32)
    w_proj = np.asarray(w_proj, np.float32)
    b_proj = np.asarray(b_proj, np.float32)

    wqT = np.ascontiguousarray(w_qkv[0:D].T).astype(mm_np)
    wkT = np.ascontiguousarray(w_qkv[D:2 * D].T).astype(mm_np)
    wvT = np.ascontiguousarray(w_qkv[2 * D:3 * D].T).astype(mm_np)
    wpT = np.ascontiguousarray(w_proj.T).astype(mm_np)
    bp = np.ascontiguousarray(b_proj.reshape(DT, 128).T)
    onesc = np.ones((128, H), mm_np)

    xTs = [np.ascontiguousarray(x[b].T).astype(mm_np) for b in range(B)]
    mbs = [np.ascontiguousarray(
        np.where(mask[b], np.float32(-NB), np.float32(0.0))
        .astype(np.float32).reshape(KTF, 128).T) for b in range(B)]

    in_maps = []
    for c in range(N_CORES):
        b, qi = divmod(c, CORES_PER_B)
        q0 = qi * QS
        in_maps.append({
            "xT": xTs[b],
            "xTq": np.ascontiguousarray(xTs[b][:, q0:q0 + QS]),
            "wqT": wqT, "wkT": wkT, "wvT": wvT, "wpT": wpT,
            "bp": bp, "mb": mbs[b], "onesc": onesc,
        })
    return in_maps


def _fb_kernel(x, mask, w_qkv, w_proj, b_proj):
    run = _fb_get_runner(1)
    in_maps = fb_make_in_maps(x, mask, w_qkv, w_proj, b_proj)
    results = run(in_maps)
    return assemble_output(results)
